# revision 65
# baseline (speedup 1.0000x reference)
"""Trainium2 Bass kernel for BroadcastResidualBlock.

Reference computation (per image, NHWC, H=W=19, C=256, HW=361):
    h1 = relu(bn1(x @ conv1_w + conv1_b))          # 1x1 conv = channel mix
    h2 = relu(dense(h1 over flattened board))       # spatial mix, per channel
    h3 = relu(bn2(h2 @ conv2_w + conv2_b))          # 1x1 conv
    out = x + h3

Strategy: pure data parallel over batch N=256 -> 32 images per core on 8
cores.  BN (inference) folds into the conv weights/biases on the host.

Precision/speed scheme (validated host-side to rel err ~2.2e-3):
  * conv1 and conv2 run as fp8e4 DoubleRow matmuls (0.5 cycles/row, 2
    k-tiles per instruction = 4x the bf16 MAC rate) with 3-term error
    compensation:  Ah@Bh + Al@Bh + Ah@Bl  where  Xl = q8(X*s - q8(X*s)).
    All three terms share one PSUM accumulation group (same scale).
  * the dense layer stays bf16 (K=361 means DoubleRow padding cancels
    its gain); dense_w is pre-scaled by s_h2/(s_x*s_w1) on the host so
    psum2 lands directly in h2's fp8 scale.
  * h2's hi/lo fp8 split is computed on device: ACT relu -> h2h8, then
    one DVE scalar_tensor_tensor (max, subtract) -> h2l8.
  * the s3 epilogue is a single DVE op: relu(psum3) + x' where the
    residual x' = xh8 + xl8 is reconstructed on the gpsimd engine
    (s_w2*s_h2 == s_x so the units line up exactly); output stores as
    fp16 and the host multiplies by 1/s_x when unmarshalling.  x ships
    once as the fp8 hi/lo pair (2B/elem), partition-major so each load
    is one contiguous >=512B run per partition (under 512B the DMA
    model charges 2x).

Matmul layouts (contraction over partitions, zero on-device transposes):
    s1: ps1[r, d]  += x8[c(kt), r_chunk].T  @ w1[c(kt), d]      (DoubleRow)
    s2: ps2[c, q]  += h1[p_chunk, c_chunk].T @ dw'[p_chunk, q]  (bf16)
    s3: ps3[d, q]  += w2[c(kt), d_chunk].T  @ h2[c(kt), q]      (DoubleRow)

Schedule: 4-deep software pipeline (s1 at step i, s2 at i+1, s3 at i+3 so
the 2-op h2 epilogue chain has two steps of slack); PSUM split 2+4+2 banks
across the three stages; warm-up matmuls on scratch SBUF at t=0 start the
PE p-state ramp (0.65 -> 1.2 -> 2.4 GHz after 3us busy) before the first
DMA lands; x loads ride the sync queue, weights ride gpsimd/SWDGE, output
stores are 8-image SWDGE batches emitted 2 steps late so triggers never
head-of-line-block a queue; the last images store per-dc on sync to cut
the drain tail.
"""

import numpy as np
import ml_dtypes

import concourse.bass as bass
import concourse.mybir as mybir
import concourse.tile as tile
from concourse import bacc
from concourse.bass_utils import run_bass_kernel_spmd

N_CORES = 8
NIMG = 32            # images per core
C = 256
HW = 361             # 19*19
P = 128
EPS = 1e-3
HWP = 368            # HW padded to 16B so DoubleRow ldweights k-tile stride aligns

F32 = mybir.dt.float32
F16 = mybir.dt.float16
BF16 = mybir.dt.bfloat16
FP8 = mybir.dt.float8e4
AF = mybir.ActivationFunctionType
ALU = mybir.AluOpType
DRMODE = mybir.MatmulPerfMode.DoubleRow

E4NP = ml_dtypes.float8_e4m3
BFNP = ml_dtypes.bfloat16

# fp8 weight blob columns: w1h | w1l | w2h | w2l (each [P, 2, 256])
O_W1H, O_W1L, O_W2H, O_W2L = 0, 256, 512, 768
WB8_COLS = 1024

# DMA batches: singles at the edges (short critical path at startup/teardown),
# pairs in steady state
BATCHES = [[0], [1]] + [[i, i + 1] for i in range(2, 30, 2)] + [[30], [31]]
BMAX = 2

_prog_cache = {}

# per-step emission order of matmul groups: (stage, group).  s1 early (its
# relu must land before next step's s2), s2 late (its h2 hi/lo epilogue
# chain gets 2 steps of slack before s3 at step+3 consumes it).
STEP_ORDER = [(1, 0), (1, 1), (1, 2), (2, 0), (3, 0), (3, 1), (2, 1)]
TAIL_ORDER = [(3, 0), (3, 1), (2, 0), (2, 1), (1, 0), (1, 1), (1, 2)]
N_WARM = 24          # warm-up matmuls (PE p-state ramp + startup gap fill)
SGRP = 8             # images per store trigger (amortizes SWDGE gen on Pool)


def build_program(has_b1: bool, has_b2: bool, has_b3: bool, reps: int = 1):
    nc = bacc.Bacc("TRN2", target_bir_lowering=False, debug=False)

    # x8: hi/lo fp8 pair, C-layout with c split as (ktile, partition)
    # partition-major so each batch is one >=512B contiguous run per ci
    x8 = nc.dram_tensor("x8", [P, NIMG, 2, 2, HWP], FP8, kind="ExternalInput").ap()
    wb8 = nc.dram_tensor("wb8", [P, 2, WB8_COLS], FP8, kind="ExternalInput").ap()
    dwb = nc.dram_tensor("dwb", [P, 3, HW], BF16, kind="ExternalInput").ap()
    b1 = b2 = b3 = None
    if has_b1:
        b1 = nc.dram_tensor("b1", [P, 3 * C], F32, kind="ExternalInput").ap()
    if has_b2:
        b2 = nc.dram_tensor("b2", [P, 2, HW], F32, kind="ExternalInput").ap()
    if has_b3:
        b3 = nc.dram_tensor("b3", [2, P], F32, kind="ExternalInput").ap()
    yc = nc.dram_tensor("yc", [NIMG, 2, P, HW], F16, kind="ExternalOutput").ap()

    batch_of = {}
    for bi, imgs in enumerate(BATCHES):
        for k, i in enumerate(imgs):
            batch_of[i] = (bi, k)

    with tile.TileContext(nc) as tc:
        with (
            tc.tile_pool(name="const", bufs=1) as cpool,
            tc.tile_pool(name="x8p", bufs=9) as x8_pool,
            tc.tile_pool(name="xrc", bufs=8) as xrec_pool,
            tc.tile_pool(name="h1", bufs=4) as h1_pool,
            tc.tile_pool(name="h2", bufs=10) as h2_pool,
            tc.tile_pool(name="yo", bufs=2) as yo_pool,
            tc.tile_pool(name="ps1", bufs=1, space="PSUM") as ps1_pool,
            tc.tile_pool(name="ps2", bufs=2, space="PSUM") as ps2_pool,
            tc.tile_pool(name="ps3", bufs=1, space="PSUM") as ps3_pool,
        ):
            # --- warm-up: start the PE ramp before any DMA lands ---------
            warm = cpool.tile([P, P], BF16)
            nc.vector.memset(warm[:], 0.0)
            pw = ps3_pool.tile([P, 1024], F32, tag="ps3", name="pw")
            for wi in range(N_WARM):
                nc.tensor.matmul(
                    pw[:, :P], warm[:], warm[:],
                    start=True, stop=True, skip_group_check=True)

            # --- constants ----------------------------------------------
            # weight blobs ride the (otherwise idle at startup) gpsimd/SWDGE
            # queue; dw goes first on the scalar queue.  This keeps each
            # queue's ~1.3us per-DMA config cost off the critical path.
            wsb8 = cpool.tile([P, 2, WB8_COLS], FP8)
            nc.gpsimd.dma_start(wsb8[:, :, : 2 * C], wb8[:, :, : 2 * C])
            nc.gpsimd.dma_start(wsb8[:, :, 2 * C :], wb8[:, :, 2 * C :])
            dwsb = cpool.tile([P, 3, HW], BF16)
            nc.scalar.dma_start(dwsb[:], dwb)

            def w1h_ap():
                return wsb8[:, :, O_W1H : O_W1H + C]

            def w1l_ap():
                return wsb8[:, :, O_W1L : O_W1L + C]

            def w2_ap(hi, dc):
                o = (O_W2H if hi else O_W2L) + dc * P
                return wsb8[:, :, o : o + P]

            b1sb = b2sb = b3sb = None
            if has_b1:
                b1sb = cpool.tile([P, 3 * C], F32)
                nc.sync.dma_start(b1sb[:], b1)
            if has_b2:
                b2sb = cpool.tile([P, 2, HW], F32)
                nc.sync.dma_start(b2sb[:], b2)
            if has_b3:
                b3sb = cpool.tile([P, 2], F32)
                nc.sync.dma_start(b3sb[:], b3.rearrange("co ci -> ci co"))

            def emit_load(bi):
                imgs = BATCHES[bi]
                nb = len(imgs)
                i0 = imgs[0]
                # tile dims: (ci, img, ktile, hi/lo, q) so the DoubleRow
                # lhsT k-tile stride is 736B (16B-aligned) and the DMA is one
                # contiguous 2944B run per partition (no <512B descriptor
                # penalty)
                x8t = x8_pool.tile([P, BMAX, 2, 2, HWP], FP8, tag="x8", name="x8t")
                nc.sync.dma_start(x8t[:, :nb], x8[:, i0 : i0 + nb])
                return x8t

            def emit_s1_group(i, k, x8t, rc, h1, pss):
                m = 128 if rc < 2 else 105
                ps = pss["s1"]
                out = ps[:m, rc * C : rc * C + C]
                lo = rc * 128
                xh = x8t[:, k, :, 0, lo : lo + m]
                xl = x8t[:, k, :, 1, lo : lo + m]
                nc.tensor.matmul(out, xh, w1h_ap(), start=True, stop=False,
                                 perf_mode=DRMODE)
                nc.tensor.matmul(out, xh, w1l_ap(), start=False, stop=False,
                                 perf_mode=DRMODE)
                nc.tensor.matmul(out, xl, w1h_ap(), start=False, stop=True,
                                 perf_mode=DRMODE)
                if rc < 2:
                    return
                # fused epilogue over all three rc slices
                if b1sb is not None:
                    nc.vector.scalar_tensor_tensor(
                        ps[:, : 3 * C], ps[:, : 3 * C], 0.0, b1sb[:],
                        ALU.bypass, ALU.add)
                nc.scalar.activation(
                    h1[:].rearrange("p a b -> p (a b)"), ps[:, : 3 * C], AF.Relu)

            def emit_s2_group(i, h1, cc, h2h, h2l, pss, split_epi=False):
                ps = pss["s2"]
                out = ps[:, cc * 512 : cc * 512 + HW]
                for pc in range(3):
                    kk = 128 if pc < 2 else 105
                    nc.tensor.matmul(
                        out,
                        h1[:kk, pc, cc * 128 : (cc + 1) * 128],
                        dwsb[:kk, pc, :],
                        start=(pc == 0),
                        stop=(pc == 2),
                    )
                if split_epi:
                    # per-cc hi/lo epilogue: shortens the h2 -> s3 chain at
                    # the pipeline tail (hi on ACT so DVE is free for the
                    # tail stores)
                    pscc = ps[:, cc * 512 : cc * 512 + HW]
                    if b2sb is not None:
                        nc.vector.scalar_tensor_tensor(
                            pscc, pscc, 0.0, b2sb[:, cc], ALU.bypass, ALU.add)
                    nc.scalar.activation(h2h[:, cc, :], pscc, AF.Relu)
                    nc.vector.scalar_tensor_tensor(
                        h2l[:, cc, :], pscc, 0.0, h2h[:, cc, :],
                        ALU.max, ALU.subtract)
                    return
                if cc == 0:
                    return
                psv = ps.rearrange("p (c x) -> p c x", c=2)[:, :, :HW]
                if b2sb is not None:
                    nc.vector.scalar_tensor_tensor(
                        psv, psv, 0.0, b2sb[:], ALU.bypass, ALU.add)
                # hi/lo fp8 split of h2: relu -> hi8 on ACT, then one DVE
                # scalar_tensor_tensor (max, subtract) -> lo8
                nc.scalar.activation(h2h[:], psv, AF.Relu)
                nc.vector.scalar_tensor_tensor(
                    h2l[:], psv, 0.0, h2h[:], ALU.max, ALU.subtract)

            def emit_s3_group(i, k, xrec, yo, h2h, h2l, dc, pss):
                ps = pss["s3"]
                slot = i % SGRP
                out = ps[:, dc * 512 : dc * 512 + HW]
                nc.tensor.matmul(out, w2_ap(True, dc), h2h[:], start=True,
                                 stop=False, perf_mode=DRMODE)
                nc.tensor.matmul(out, w2_ap(True, dc), h2l[:], start=False,
                                 stop=False, perf_mode=DRMODE)
                nc.tensor.matmul(out, w2_ap(False, dc), h2h[:], start=False,
                                 stop=True, perf_mode=DRMODE)
                if i >= NIMG - 2 and b3sb is None:
                    # tail: per-dc DVE epilogue + immediate store so the
                    # final store chain overlaps the other half's matmuls
                    pscc = ps[:, dc * 512 : dc * 512 + HW]
                    nc.vector.scalar_tensor_tensor(
                        yo[:, slot, dc, :], pscc, 0.0, xrec[:, dc, :],
                        ALU.max, ALU.add)
                    nc.sync.dma_start(yc[i, dc], yo[:, slot, dc, :])
                    return
                if dc == 0:
                    return
                psv = ps.rearrange("p (c x) -> p c x", c=2)[:, :, :HW]
                xap = xrec[:]
                if b3sb is not None:
                    for d2 in range(2):
                        nc.scalar.activation(
                            yo[:, slot, d2, :], psv[:, d2, :], AF.Relu,
                            bias=b3sb[:, d2 : d2 + 1])
                    nc.vector.tensor_add(yo[:, slot, :, :], yo[:, slot, :, :], xap)
                    if i >= NIMG - 2:
                        nc.sync.dma_start(
                            yc[i].rearrange("co ci q -> ci co q"), yo[:, slot])
                else:
                    # single fused DVE epilogue: relu(psum3) + x'
                    nc.vector.scalar_tensor_tensor(
                        yo[:, slot, :, :], psv, 0.0, xap, ALU.max, ALU.add)

            def emit_store(g0, yo, n):
                # SWDGE store, emitted well after the data is written so the
                # trigger never head-of-line-blocks Pool's residual adds
                s0 = g0 % SGRP
                nc.gpsimd.dma_start(
                    yc[g0 : g0 + n].rearrange("n co ci q -> ci n co q"),
                    yo[:, s0 : s0 + n])

            def body():
                # software pipeline: s1(i) | s2(i-1) | s3(i-2), interleaved at
                # matmul-group granularity so PSUM slot releases stagger
                xs, h1s, h2s, yos, xrecs = {}, {}, {}, {}, {}

                def load_batch(bi):
                    x8t = emit_load(bi)
                    for k, i in enumerate(BATCHES[bi]):
                        xs[i] = (x8t, k)

                loaded = 0
                for pb in range(4):
                    load_batch(pb)
                    loaded += 1
                for step in range(NIMG + 3):
                    if step % 2 == 0 and loaded < len(BATCHES):
                        load_batch(loaded)
                        loaded += 1
                    i1 = step if step < NIMG else None
                    i2 = step - 1 if 1 <= step <= NIMG else None
                    i3 = step - 3 if step >= 3 else None
                    pss = {}
                    if i1 is not None:
                        pss["s1"] = ps1_pool.tile([P, 768], F32, tag="ps1", name="ps1")
                    if i3 is not None:
                        # last image borrows a (by then idle) ps2 slot so its
                        # matmuls don't WAR-wait on img30's epilogue
                        pool3 = ps2_pool if i3 == NIMG - 1 else ps3_pool
                        tag3 = "ps2" if i3 == NIMG - 1 else "ps3"
                        pss["s3"] = pool3.tile([P, 1024], F32, tag=tag3, name="ps3")
                    if i2 is not None:
                        pss["s2"] = ps2_pool.tile([P, 1024], F32, tag="ps2", name="ps2")
                    if i1 is not None:
                        h1s[i1] = h1_pool.tile([P, 3, C], BF16, tag="h1", name="h1")
                    if i1 is not None:
                        # residual reconstruction on the otherwise idle
                        # gpsimd engine: xrec = xh8 + xl8 (= s_x * x exactly
                        # in epilogue units, since s_w2*s_h2 == s_x)
                        x8t, k = xs[i1]
                        xrec = xrec_pool.tile([P, 2, HW], F16, tag="xc", name="xrec")
                        nc.gpsimd.tensor_add(
                            xrec[:], x8t[:, k, :, 0, :HW], x8t[:, k, :, 1, :HW])
                        xrecs[i1] = xrec
                    if i3 is not None and i3 % SGRP == 0:
                        yos[i3 // SGRP] = yo_pool.tile(
                            [P, SGRP, 2, HW], F16, tag="yo", name="yo")
                    if i2 is not None:
                        h2s[i2] = (
                            h2_pool.tile([P, 2, HW], FP8, tag="h2h", name="h2h"),
                            h2_pool.tile([P, 2, HW], FP8, tag="h2l", name="h2l"),
                        )
                    order = STEP_ORDER if step < NIMG else TAIL_ORDER
                    for stg, g in order:
                        if stg == 1 and i1 is not None:
                            x8t, k = xs[i1]
                            emit_s1_group(i1, k, x8t, g, h1s[i1], pss)
                        elif stg == 3 and i3 is not None:
                            emit_s3_group(i3, 0, xrecs[i3], yos[i3 // SGRP],
                                          h2s[i3][0], h2s[i3][1], g, pss)
                        elif stg == 2 and i2 is not None:
                            emit_s2_group(i2, h1s[i2], g,
                                          h2s[i2][0], h2s[i2][1], pss,
                                          split_epi=(i2 >= NIMG - 2))
                    if i2 is not None:
                        h1s.pop(i2)
                    if i3 is not None:
                        h2s.pop(i3)
                        xrecs.pop(i3)
                        # delayed stores: 8-img groups for 0..23 (2 steps
                        # after the group's last s3), pairs for 24..29
                        if i3 in (9, 17, 25):
                            g0 = i3 - 9
                            emit_store(g0, yos[g0 // SGRP], SGRP)
                        elif i3 == 28:
                            emit_store(24, yos[3], 2)
                        elif i3 == 30:
                            emit_store(26, yos[3], 2)
                        elif i3 == NIMG - 1:
                            emit_store(28, yos[3], 2)

            if reps == 1:
                body()
            else:
                with tc.For_i(0, reps, 1):
                    body()

    nc.compile()
    return nc


def _get_program(key):
    if key not in _prog_cache:
        _prog_cache[key] = build_program(*key)
    return _prog_cache[key]


def _pow2floor(v):
    return 2.0 ** np.floor(np.log2(max(float(v), 1e-30)))


def _q8(a):
    return np.clip(a, -240.0, 240.0).astype(E4NP)


def _marshal(x, conv1_w, conv1_b, bn1_mean, bn1_var, bn1_beta,
             dense_w, dense_b, conv2_w, conv2_b, bn2_mean, bn2_var, bn2_beta):
    n = x.shape[0]
    rs1 = 1.0 / np.sqrt(bn1_var.astype(np.float64) + EPS)
    rs2 = 1.0 / np.sqrt(bn2_var.astype(np.float64) + EPS)
    w1f = conv1_w.astype(np.float64) * rs1[None, :]
    w2f = conv2_w.astype(np.float64) * rs2[None, :]
    b1f = (conv1_b - bn1_mean).astype(np.float64) * rs1 + bn1_beta
    b2f = dense_b.astype(np.float64)
    b3f = (conv2_b - bn2_mean).astype(np.float64) * rs2 + bn2_beta
    has_b1 = bool(np.any(b1f != 0.0))
    has_b2 = bool(np.any(b2f != 0.0))
    has_b3 = bool(np.any(b3f != 0.0))

    s_x = _pow2floor(224.0 / max(np.abs(x).max(), 1e-20))
    s_w1 = _pow2floor(224.0 / max(np.abs(w1f).max(), 1e-20))
    s_h2 = 4.0
    # s_w2*s_h2 == s_x so the residual in epilogue units is exactly xh8+xl8
    # (reconstructed on device); w2 values are small so s_x/4 keeps them
    # well inside e4m3 range
    s_w2 = s_x / s_h2
    k_out = 1.0 / (s_w2 * s_h2)

    # x in C-layout [n, C, HW], C split as (ktile, partition)
    xq = np.ascontiguousarray(
        x.reshape(n, HW, C).transpose(0, 2, 1)).astype(np.float32)
    xh8 = _q8(xq * np.float32(s_x))
    xl8 = _q8(xq * np.float32(s_x) - xh8.astype(np.float32))
    x8_all = np.stack([xh8, xl8], axis=1)        # [n, t, C, HW]
    x8_all = x8_all.reshape(N_CORES, NIMG, 2, 2, P, HW)
    x8_pad = np.zeros((N_CORES, NIMG, 2, 2, P, HWP), E4NP)
    x8_pad[..., :HW] = x8_all
    # -> [core, ci, img, kt, t, q]
    x8_all = np.ascontiguousarray(x8_pad.transpose(0, 4, 1, 3, 2, 5))

    # fp8 weight blob [P, 2, 1024]: w1h | w1l | w2h | w2l
    blob = np.zeros((P, 2, WB8_COLS), E4NP)
    w1h = _q8(w1f * s_w1)
    w1l = _q8(w1f * s_w1 - w1h.astype(np.float64))
    w2h = _q8(w2f * s_w2)
    w2l = _q8(w2f * s_w2 - w2h.astype(np.float64))
    for kt in range(2):
        blob[:, kt, O_W1H : O_W1H + C] = w1h[kt * P : (kt + 1) * P]
        blob[:, kt, O_W1L : O_W1L + C] = w1l[kt * P : (kt + 1) * P]
        blob[:, kt, O_W2H : O_W2H + C] = w2h[kt * P : (kt + 1) * P]
        blob[:, kt, O_W2L : O_W2L + C] = w2l[kt * P : (kt + 1) * P]

    # dense weight, pre-scaled so psum2 = s_h2 * h2pre, padded to 3 k-tiles
    dwf = dense_w.astype(np.float64) * (s_h2 / (s_x * s_w1))
    dwp = np.zeros((3 * P, HW), np.float64)
    dwp[:HW] = dwf
    dwb = np.ascontiguousarray(dwp.reshape(3, P, HW).transpose(1, 0, 2)).astype(BFNP)

    in_maps = []
    for c in range(N_CORES):
        m = {"x8": x8_all[c], "wb8": blob, "dwb": dwb}
        if has_b1:
            m["b1"] = np.ascontiguousarray(np.broadcast_to(
                np.tile(b1f * (s_x * s_w1), 3).astype(np.float32), (P, 3 * C)))
        if has_b2:
            m["b2"] = np.ascontiguousarray(np.broadcast_to(
                (b2f * s_h2).astype(np.float32), (P, 2, HW)))
        if has_b3:
            m["b3"] = np.ascontiguousarray(
                (b3f * (s_w2 * s_h2)).astype(np.float32).reshape(2, P))
        in_maps.append(m)
    return (has_b1, has_b2, has_b3), in_maps, k_out


def _unmarshal(results, n, h, w, k_out):
    y = np.stack([results[c]["yc"] for c in range(N_CORES)])
    y = y.astype(np.float32).reshape(n, C, HW).transpose(0, 2, 1)
    y = y * np.float32(k_out)
    return np.ascontiguousarray(y.reshape(n, h, w, C))


def kernel(x, conv1_w, conv1_b, bn1_mean, bn1_var, bn1_beta,
           dense_w, dense_b, conv2_w, conv2_b, bn2_mean, bn2_var, bn2_beta):
    n, h, w, _ = x.shape
    flags, in_maps, k_out = _marshal(
        x, conv1_w, conv1_b, bn1_mean, bn1_var, bn1_beta,
        dense_w, dense_b, conv2_w, conv2_b, bn2_mean, bn2_var, bn2_beta)
    nc = _get_program((*flags, 1))
    res = run_bass_kernel_spmd(nc, in_maps, list(range(N_CORES)))
    return _unmarshal(res.results, n, h, w, k_out)


# revision 84
# speedup vs baseline: 1.0184x; 1.0184x over previous
"""Trainium2 Bass kernel for BroadcastResidualBlock.

Reference computation (per image, NHWC, H=W=19, C=256, HW=361):
    h1 = relu(bn1(x @ conv1_w + conv1_b))          # 1x1 conv = channel mix
    h2 = relu(dense(h1 over flattened board))       # spatial mix, per channel
    h3 = relu(bn2(h2 @ conv2_w + conv2_b))          # 1x1 conv
    out = x + h3

Strategy: pure data parallel over batch N=256 -> 32 images per core on 8
cores.  BN (inference) folds into the conv weights/biases on the host.

Precision/speed scheme (validated host-side to rel err ~2.2e-3):
  * conv1 and conv2 run as fp8e4 DoubleRow matmuls (0.5 cycles/row, 2
    k-tiles per instruction = 4x the bf16 MAC rate) with 3-term error
    compensation:  Ah@Bh + Al@Bh + Ah@Bl  where  Xl = q8(X*s - q8(X*s)).
    All three terms share one PSUM accumulation group (same scale).
  * the dense layer stays bf16 (K=361 means DoubleRow padding cancels
    its gain); dense_w is pre-scaled by s_h2/(s_x*s_w1) on the host so
    psum2 lands directly in h2's fp8 scale.
  * h2's hi/lo fp8 split is computed on device: ACT relu -> h2h8, then
    one DVE scalar_tensor_tensor (max, subtract) -> h2l8.
  * the s3 epilogue is a single DVE op: relu(psum3) + x' where the
    residual x' = xh8 + xl8 is reconstructed on the gpsimd engine
    (s_w2*s_h2 == s_x so the units line up exactly); output stores as
    fp16 and the host multiplies by 1/s_x when unmarshalling.  x ships
    once as the fp8 hi/lo pair (2B/elem), partition-major so each load
    is one contiguous >=512B run per partition (under 512B the DMA
    model charges 2x).

Matmul layouts (contraction over partitions, zero on-device transposes):
    s1: ps1[r, d]  += x8[c(kt), r_chunk].T  @ w1[c(kt), d]      (DoubleRow)
    s2: ps2[c, q]  += h1[p_chunk, c_chunk].T @ dw'[p_chunk, q]  (bf16)
    s3: ps3[d, q]  += w2[c(kt), d_chunk].T  @ h2[c(kt), q]      (DoubleRow)

Schedule: 4-deep software pipeline (s1 at step i, s2 at i+1, s3 at i+3 so
the 2-op h2 epilogue chain has two steps of slack); PSUM split 2+4+2 banks
across the three stages; warm-up matmuls on scratch SBUF at t=0 start the
PE p-state ramp (0.65 -> 1.2 -> 2.4 GHz after 3us busy) before the first
DMA lands; x loads ride the sync queue, weights ride gpsimd/SWDGE, output
stores are 8-image SWDGE batches emitted 2 steps late so triggers never
head-of-line-block a queue; the last images store per-dc on sync to cut
the drain tail.
"""

import numpy as np
import ml_dtypes

import concourse.bass as bass
import concourse.mybir as mybir
import concourse.tile as tile
from concourse import bacc
from concourse.bass_utils import run_bass_kernel_spmd

N_CORES = 8
NIMG = 32            # images per core
C = 256
HW = 361             # 19*19
P = 128
EPS = 1e-3
HWP = 368            # HW padded to 16B so DoubleRow ldweights k-tile stride aligns

F32 = mybir.dt.float32
F16 = mybir.dt.float16
BF16 = mybir.dt.bfloat16
FP8 = mybir.dt.float8e4
AF = mybir.ActivationFunctionType
ALU = mybir.AluOpType
DRMODE = mybir.MatmulPerfMode.DoubleRow

E4NP = ml_dtypes.float8_e4m3
BFNP = ml_dtypes.bfloat16

# fp8 weight blob columns: w1h | w1l | w2h | w2l (each [P, 2, 256])
O_W1H, O_W1L, O_W2H, O_W2L = 0, 256, 512, 768
WB8_COLS = 1024

# DMA batches: singles at the edges (short critical path at startup/teardown),
# pairs in steady state
BATCHES = [[0], [1]] + [[i, i + 1] for i in range(2, 30, 2)] + [[30], [31]]
BMAX = 2

_prog_cache = {}

# per-step emission order of matmul groups: (stage, group).  s1 early (its
# relu must land before next step's s2), s2 late (its h2 hi/lo epilogue
# chain gets 2 steps of slack before s3 at step+3 consumes it).
STEP_ORDER = [(1, 0), (1, 1), (1, 2), (2, 0), (3, 0), (3, 1), (2, 1)]
TAIL_ORDER = [(2, 0), (2, 1), (3, 0), (3, 1), (1, 0), (1, 1), (1, 2)]
N_WARM = 24          # warm-up matmuls (PE p-state ramp + startup gap fill)
SGRP = 8             # images per store trigger (amortizes SWDGE gen on Pool)


def build_program(has_b1: bool, has_b2: bool, has_b3: bool, reps: int = 1):
    nc = bacc.Bacc("TRN2", target_bir_lowering=False, debug=False)

    # x8: hi/lo fp8 pair, C-layout with c split as (ktile, partition)
    # partition-major so each batch is one >=512B contiguous run per ci
    x8 = nc.dram_tensor("x8", [P, NIMG, 2, 2, HWP], FP8, kind="ExternalInput").ap()
    wb8 = nc.dram_tensor("wb8", [P, 2, WB8_COLS], FP8, kind="ExternalInput").ap()
    dwb = nc.dram_tensor("dwb", [P, 3, HW], BF16, kind="ExternalInput").ap()
    wf2 = nc.dram_tensor("wf2", [P, 2, C], F16, kind="ExternalInput").ap()
    b1 = b2 = b3 = None
    if has_b1:
        b1 = nc.dram_tensor("b1", [P, 3 * C], F32, kind="ExternalInput").ap()
    if has_b2:
        b2 = nc.dram_tensor("b2", [P, 2, HW], F32, kind="ExternalInput").ap()
    if has_b3:
        b3 = nc.dram_tensor("b3", [2, P], F32, kind="ExternalInput").ap()
    yc = nc.dram_tensor("yc", [NIMG, 2, P, HW], F16, kind="ExternalOutput").ap()

    batch_of = {}
    for bi, imgs in enumerate(BATCHES):
        for k, i in enumerate(imgs):
            batch_of[i] = (bi, k)

    with tile.TileContext(nc) as tc:
        with (
            tc.tile_pool(name="const", bufs=1) as cpool,
            tc.tile_pool(name="x8p", bufs=9) as x8_pool,
            tc.tile_pool(name="xrc", bufs=8) as xrec_pool,
            tc.tile_pool(name="h1", bufs=4) as h1_pool,
            tc.tile_pool(name="h2", bufs=10) as h2_pool,
            tc.tile_pool(name="yo", bufs=2) as yo_pool,
            tc.tile_pool(name="ps1", bufs=1, space="PSUM") as ps1_pool,
            tc.tile_pool(name="ps2", bufs=2, space="PSUM") as ps2_pool,
            tc.tile_pool(name="ps3", bufs=1, space="PSUM") as ps3_pool,
        ):
            # --- warm-up: start the PE ramp before any DMA lands ---------
            warm = cpool.tile([P, P], BF16)
            nc.vector.memset(warm[:], 0.0)
            pw = ps3_pool.tile([P, 1024], F32, tag="ps3", name="pw")
            for wi in range(N_WARM):
                nc.tensor.matmul(
                    pw[:, :P], warm[:], warm[:],
                    start=True, stop=True, skip_group_check=True)

            # --- constants ----------------------------------------------
            # weight blobs ride the (otherwise idle at startup) gpsimd/SWDGE
            # queue; dw goes first on the scalar queue.  This keeps each
            # queue's ~1.3us per-DMA config cost off the critical path.
            wsb8 = cpool.tile([P, 2, WB8_COLS], FP8)
            nc.gpsimd.dma_start(wsb8[:, :, : 2 * C], wb8[:, :, : 2 * C])
            nc.gpsimd.dma_start(wsb8[:, :, 2 * C :], wb8[:, :, 2 * C :])
            dwsb = cpool.tile([P, 3, HW], BF16)
            nc.scalar.dma_start(dwsb[:], dwb)
            # fp16 copy of w2 for the tail images' plain-matmul s3 path
            wf2sb = cpool.tile([P, 2, C], F16)
            nc.scalar.dma_start(wf2sb[:], wf2)

            def w1h_ap():
                return wsb8[:, :, O_W1H : O_W1H + C]

            def w1l_ap():
                return wsb8[:, :, O_W1L : O_W1L + C]

            def w2_ap(hi, dc):
                o = (O_W2H if hi else O_W2L) + dc * P
                return wsb8[:, :, o : o + P]

            b1sb = b2sb = b3sb = None
            if has_b1:
                b1sb = cpool.tile([P, 3 * C], F32)
                nc.sync.dma_start(b1sb[:], b1)
            if has_b2:
                b2sb = cpool.tile([P, 2, HW], F32)
                nc.sync.dma_start(b2sb[:], b2)
            if has_b3:
                b3sb = cpool.tile([P, 2], F32)
                nc.sync.dma_start(b3sb[:], b3.rearrange("co ci -> ci co"))

            def emit_load(bi):
                imgs = BATCHES[bi]
                nb = len(imgs)
                i0 = imgs[0]
                # tile dims: (ci, img, ktile, hi/lo, q) so the DoubleRow
                # lhsT k-tile stride is 736B (16B-aligned) and the DMA is one
                # contiguous 2944B run per partition (no <512B descriptor
                # penalty)
                x8t = x8_pool.tile([P, BMAX, 2, 2, HWP], FP8, tag="x8", name="x8t")
                nc.sync.dma_start(x8t[:, :nb], x8[:, i0 : i0 + nb])
                return x8t

            def emit_s1_group(i, k, x8t, rc, h1, pss):
                m = 128 if rc < 2 else 105
                ps = pss["s1"]
                out = ps[:m, rc * C : rc * C + C]
                lo = rc * 128
                xh = x8t[:, k, :, 0, lo : lo + m]
                xl = x8t[:, k, :, 1, lo : lo + m]
                nc.tensor.matmul(out, xh, w1h_ap(), start=True, stop=False,
                                 perf_mode=DRMODE)
                nc.tensor.matmul(out, xh, w1l_ap(), start=False, stop=False,
                                 perf_mode=DRMODE)
                nc.tensor.matmul(out, xl, w1h_ap(), start=False, stop=True,
                                 perf_mode=DRMODE)
                if rc < 2:
                    return
                # fused epilogue over all three rc slices
                if b1sb is not None:
                    nc.vector.scalar_tensor_tensor(
                        ps[:, : 3 * C], ps[:, : 3 * C], 0.0, b1sb[:],
                        ALU.bypass, ALU.add)
                nc.scalar.activation(
                    h1[:].rearrange("p a b -> p (a b)"), ps[:, : 3 * C], AF.Relu)

            def emit_s2_group(i, h1, cc, h2h, h2l, pss, split_epi=False):
                ps = pss["s2"]
                out = ps[:, cc * 512 : cc * 512 + HW]
                for pc in range(3):
                    kk = 128 if pc < 2 else 105
                    nc.tensor.matmul(
                        out,
                        h1[:kk, pc, cc * 128 : (cc + 1) * 128],
                        dwsb[:kk, pc, :],
                        start=(pc == 0),
                        stop=(pc == 2),
                    )
                if split_epi:
                    # tail images: h2 goes to a single fused fp16 tensor (no
                    # hi/lo split, one ACT op) so the drain-critical chain is
                    # s2 -> one relu -> plain fp16 s3, with DVE free for the
                    # final epilogues
                    if cc == 0:
                        return
                    psv2 = ps.rearrange("p (c x) -> p c x", c=2)[:, :, :HW]
                    if b2sb is not None:
                        nc.vector.scalar_tensor_tensor(
                            psv2, psv2, 0.0, b2sb[:], ALU.bypass, ALU.add)
                    nc.scalar.activation(h2h[:], psv2, AF.Relu)
                    return
                if cc == 0:
                    return
                psv = ps.rearrange("p (c x) -> p c x", c=2)[:, :, :HW]
                if b2sb is not None:
                    nc.vector.scalar_tensor_tensor(
                        psv, psv, 0.0, b2sb[:], ALU.bypass, ALU.add)
                # hi/lo fp8 split of h2: relu -> hi8 on ACT, then one DVE
                # scalar_tensor_tensor (max, subtract) -> lo8
                nc.scalar.activation(h2h[:], psv, AF.Relu)
                nc.vector.scalar_tensor_tensor(
                    h2l[:], psv, 0.0, h2h[:], ALU.max, ALU.subtract)

            def emit_s3_group(i, k, xrec, yo, h2h, h2l, dc, pss):
                ps = pss["s3"]
                slot = i % SGRP
                out = ps[:, dc * 512 : dc * 512 + HW]
                if h2l is None:
                    # tail image: plain fp16 matmuls off the unsplit h2
                    for kt in range(2):
                        nc.tensor.matmul(
                            out, wf2sb[:, kt, dc * P : dc * P + P],
                            h2h[:, kt, :], start=(kt == 0), stop=(kt == 1))
                else:
                    nc.tensor.matmul(out, w2_ap(True, dc), h2h[:], start=True,
                                     stop=False, perf_mode=DRMODE)
                    nc.tensor.matmul(out, w2_ap(True, dc), h2l[:], start=False,
                                     stop=False, perf_mode=DRMODE)
                    nc.tensor.matmul(out, w2_ap(False, dc), h2h[:], start=False,
                                     stop=True, perf_mode=DRMODE)
                if i >= NIMG - 2 and b3sb is None:
                    # tail: per-dc DVE epilogue, one store per image
                    pscc = ps[:, dc * 512 : dc * 512 + HW]
                    nc.vector.scalar_tensor_tensor(
                        yo[:, slot, dc, :], pscc, 0.0, xrec[:, dc, :],
                        ALU.max, ALU.add)
                    if dc == 1:
                        nc.sync.dma_start(
                            yc[i].rearrange("co ci q -> ci co q"),
                            yo[:, slot])
                    return
                if dc == 0:
                    return
                psv = ps.rearrange("p (c x) -> p c x", c=2)[:, :, :HW]
                xap = xrec[:]
                if b3sb is not None:
                    for d2 in range(2):
                        nc.scalar.activation(
                            yo[:, slot, d2, :], psv[:, d2, :], AF.Relu,
                            bias=b3sb[:, d2 : d2 + 1])
                    nc.vector.tensor_add(yo[:, slot, :, :], yo[:, slot, :, :], xap)
                    if i >= NIMG - 2:
                        nc.sync.dma_start(
                            yc[i].rearrange("co ci q -> ci co q"), yo[:, slot])
                else:
                    # single fused DVE epilogue: relu(psum3) + x'
                    nc.vector.scalar_tensor_tensor(
                        yo[:, slot, :, :], psv, 0.0, xap, ALU.max, ALU.add)

            def emit_store(g0, yo, n):
                # SWDGE store, emitted well after the data is written so the
                # trigger never head-of-line-blocks Pool's residual adds
                s0 = g0 % SGRP
                nc.gpsimd.dma_start(
                    yc[g0 : g0 + n].rearrange("n co ci q -> ci n co q"),
                    yo[:, s0 : s0 + n])

            def body():
                # software pipeline: s1(i) | s2(i-1) | s3(i-2), interleaved at
                # matmul-group granularity so PSUM slot releases stagger
                xs, h1s, h2s, yos, xrecs = {}, {}, {}, {}, {}

                def load_batch(bi):
                    x8t = emit_load(bi)
                    for k, i in enumerate(BATCHES[bi]):
                        xs[i] = (x8t, k)

                loaded = 0
                for pb in range(4):
                    load_batch(pb)
                    loaded += 1
                for step in range(NIMG + 3):
                    if step % 2 == 0 and loaded < len(BATCHES):
                        load_batch(loaded)
                        loaded += 1
                    i1 = step if step < NIMG else None
                    i2 = step - 1 if 1 <= step <= NIMG else None
                    i3 = step - 3 if step >= 3 else None
                    pss = {}
                    if i1 is not None:
                        pss["s1"] = ps1_pool.tile([P, 768], F32, tag="ps1", name="ps1")
                    if i3 is not None:
                        # last image borrows a (by then idle) ps2 slot so its
                        # matmuls don't WAR-wait on img30's epilogue
                        pool3 = ps2_pool if i3 == NIMG - 1 else ps3_pool
                        tag3 = "ps2" if i3 == NIMG - 1 else "ps3"
                        pss["s3"] = pool3.tile([P, 1024], F32, tag=tag3, name="ps3")
                    if i2 is not None:
                        pss["s2"] = ps2_pool.tile([P, 1024], F32, tag="ps2", name="ps2")
                    if i1 is not None:
                        h1s[i1] = h1_pool.tile([P, 3, C], BF16, tag="h1", name="h1")
                    if i1 is not None:
                        # residual reconstruction on the otherwise idle
                        # gpsimd engine: xrec = xh8 + xl8 (= s_x * x exactly
                        # in epilogue units, since s_w2*s_h2 == s_x)
                        x8t, k = xs[i1]
                        xrec = xrec_pool.tile([P, 2, HW], F16, tag="xc", name="xrec")
                        nc.gpsimd.tensor_add(
                            xrec[:], x8t[:, k, :, 0, :HW], x8t[:, k, :, 1, :HW])
                        xrecs[i1] = xrec
                    if i3 is not None and i3 % SGRP == 0:
                        yos[i3 // SGRP] = yo_pool.tile(
                            [P, SGRP, 2, HW], F16, tag="yo", name="yo")
                    if i2 is not None:
                        if i2 >= NIMG - 3:
                            h2s[i2] = (
                                xrec_pool.tile([P, 2, HW], F16, tag="xc",
                                               name="h2f"),
                                None,
                            )
                        else:
                            h2s[i2] = (
                                h2_pool.tile([P, 2, HW], FP8, tag="h2h", name="h2h"),
                                h2_pool.tile([P, 2, HW], FP8, tag="h2l", name="h2l"),
                            )
                    order = STEP_ORDER if step < NIMG else TAIL_ORDER
                    for stg, g in order:
                        if stg == 1 and i1 is not None:
                            x8t, k = xs[i1]
                            emit_s1_group(i1, k, x8t, g, h1s[i1], pss)
                        elif stg == 3 and i3 is not None:
                            emit_s3_group(i3, 0, xrecs[i3], yos[i3 // SGRP],
                                          h2s[i3][0], h2s[i3][1], g, pss)
                        elif stg == 2 and i2 is not None:
                            emit_s2_group(i2, h1s[i2], g,
                                          h2s[i2][0], h2s[i2][1], pss,
                                          split_epi=(i2 >= NIMG - 3))
                    if i2 is not None:
                        h1s.pop(i2)
                    if i3 is not None:
                        h2s.pop(i3)
                        xrecs.pop(i3)
                        # delayed stores: 8-img groups for 0..23 (2 steps
                        # after the group's last s3), pairs for 24..29
                        if i3 in (9, 17, 25):
                            g0 = i3 - 9
                            emit_store(g0, yos[g0 // SGRP], SGRP)
                        elif i3 == 28:
                            emit_store(24, yos[3], 2)
                        elif i3 == 30:
                            emit_store(26, yos[3], 2)
                        elif i3 == NIMG - 1:
                            emit_store(28, yos[3], 2)

            if reps == 1:
                body()
            else:
                with tc.For_i(0, reps, 1):
                    body()

    nc.compile()
    return nc


def _get_program(key):
    if key not in _prog_cache:
        _prog_cache[key] = build_program(*key)
    return _prog_cache[key]


def _pow2floor(v):
    return 2.0 ** np.floor(np.log2(max(float(v), 1e-30)))


def _q8(a):
    return np.clip(a, -240.0, 240.0).astype(E4NP)


def _marshal(x, conv1_w, conv1_b, bn1_mean, bn1_var, bn1_beta,
             dense_w, dense_b, conv2_w, conv2_b, bn2_mean, bn2_var, bn2_beta):
    n = x.shape[0]
    rs1 = 1.0 / np.sqrt(bn1_var.astype(np.float64) + EPS)
    rs2 = 1.0 / np.sqrt(bn2_var.astype(np.float64) + EPS)
    w1f = conv1_w.astype(np.float64) * rs1[None, :]
    w2f = conv2_w.astype(np.float64) * rs2[None, :]
    b1f = (conv1_b - bn1_mean).astype(np.float64) * rs1 + bn1_beta
    b2f = dense_b.astype(np.float64)
    b3f = (conv2_b - bn2_mean).astype(np.float64) * rs2 + bn2_beta
    has_b1 = bool(np.any(b1f != 0.0))
    has_b2 = bool(np.any(b2f != 0.0))
    has_b3 = bool(np.any(b3f != 0.0))

    s_x = _pow2floor(224.0 / max(np.abs(x).max(), 1e-20))
    s_w1 = _pow2floor(224.0 / max(np.abs(w1f).max(), 1e-20))
    s_h2 = 4.0
    # s_w2*s_h2 == s_x so the residual in epilogue units is exactly xh8+xl8
    # (reconstructed on device); w2 values are small so s_x/4 keeps them
    # well inside e4m3 range
    s_w2 = s_x / s_h2
    k_out = 1.0 / (s_w2 * s_h2)

    # x in C-layout [n, C, HW], C split as (ktile, partition)
    xq = np.ascontiguousarray(
        x.reshape(n, HW, C).transpose(0, 2, 1)).astype(np.float32)
    xh8 = _q8(xq * np.float32(s_x))
    xl8 = _q8(xq * np.float32(s_x) - xh8.astype(np.float32))
    x8_all = np.stack([xh8, xl8], axis=1)        # [n, t, C, HW]
    x8_all = x8_all.reshape(N_CORES, NIMG, 2, 2, P, HW)
    x8_pad = np.zeros((N_CORES, NIMG, 2, 2, P, HWP), E4NP)
    x8_pad[..., :HW] = x8_all
    # -> [core, ci, img, kt, t, q]
    x8_all = np.ascontiguousarray(x8_pad.transpose(0, 4, 1, 3, 2, 5))

    # fp8 weight blob [P, 2, 1024]: w1h | w1l | w2h | w2l
    blob = np.zeros((P, 2, WB8_COLS), E4NP)
    w1h = _q8(w1f * s_w1)
    w1l = _q8(w1f * s_w1 - w1h.astype(np.float64))
    w2h = _q8(w2f * s_w2)
    w2l = _q8(w2f * s_w2 - w2h.astype(np.float64))
    for kt in range(2):
        blob[:, kt, O_W1H : O_W1H + C] = w1h[kt * P : (kt + 1) * P]
        blob[:, kt, O_W1L : O_W1L + C] = w1l[kt * P : (kt + 1) * P]
        blob[:, kt, O_W2H : O_W2H + C] = w2h[kt * P : (kt + 1) * P]
        blob[:, kt, O_W2L : O_W2L + C] = w2l[kt * P : (kt + 1) * P]

    wf2 = np.ascontiguousarray(
        (w2f * s_w2).reshape(2, P, C).transpose(1, 0, 2)).astype(np.float16)

    # dense weight, pre-scaled so psum2 = s_h2 * h2pre, padded to 3 k-tiles
    dwf = dense_w.astype(np.float64) * (s_h2 / (s_x * s_w1))
    dwp = np.zeros((3 * P, HW), np.float64)
    dwp[:HW] = dwf
    dwb = np.ascontiguousarray(dwp.reshape(3, P, HW).transpose(1, 0, 2)).astype(BFNP)

    in_maps = []
    for c in range(N_CORES):
        m = {"x8": x8_all[c], "wb8": blob, "dwb": dwb, "wf2": wf2}
        if has_b1:
            m["b1"] = np.ascontiguousarray(np.broadcast_to(
                np.tile(b1f * (s_x * s_w1), 3).astype(np.float32), (P, 3 * C)))
        if has_b2:
            m["b2"] = np.ascontiguousarray(np.broadcast_to(
                (b2f * s_h2).astype(np.float32), (P, 2, HW)))
        if has_b3:
            m["b3"] = np.ascontiguousarray(
                (b3f * (s_w2 * s_h2)).astype(np.float32).reshape(2, P))
        in_maps.append(m)
    return (has_b1, has_b2, has_b3), in_maps, k_out


def _unmarshal(results, n, h, w, k_out):
    y = np.stack([results[c]["yc"] for c in range(N_CORES)])
    y = y.astype(np.float32).reshape(n, C, HW).transpose(0, 2, 1)
    y = y * np.float32(k_out)
    return np.ascontiguousarray(y.reshape(n, h, w, C))


def kernel(x, conv1_w, conv1_b, bn1_mean, bn1_var, bn1_beta,
           dense_w, dense_b, conv2_w, conv2_b, bn2_mean, bn2_var, bn2_beta):
    n, h, w, _ = x.shape
    flags, in_maps, k_out = _marshal(
        x, conv1_w, conv1_b, bn1_mean, bn1_var, bn1_beta,
        dense_w, dense_b, conv2_w, conv2_b, bn2_mean, bn2_var, bn2_beta)
    nc = _get_program((*flags, 1))
    res = run_bass_kernel_spmd(nc, in_maps, list(range(N_CORES)))
    return _unmarshal(res.results, n, h, w, k_out)


# revision 98
# speedup vs baseline: 1.0200x; 1.0016x over previous
"""Trainium2 Bass kernel for BroadcastResidualBlock.

Reference computation (per image, NHWC, H=W=19, C=256, HW=361):
    h1 = relu(bn1(x @ conv1_w + conv1_b))          # 1x1 conv = channel mix
    h2 = relu(dense(h1 over flattened board))       # spatial mix, per channel
    h3 = relu(bn2(h2 @ conv2_w + conv2_b))          # 1x1 conv
    out = x + h3

Strategy: pure data parallel over batch N=256 -> 32 images per core on 8
cores.  BN (inference) folds into the conv weights/biases on the host.

Precision/speed scheme (validated host-side to rel err ~2.2e-3):
  * conv1 and conv2 run as fp8e4 DoubleRow matmuls (0.5 cycles/row, 2
    k-tiles per instruction = 4x the bf16 MAC rate) with 3-term error
    compensation:  Ah@Bh + Al@Bh + Ah@Bl  where  Xl = q8(X*s - q8(X*s)).
    All three terms share one PSUM accumulation group (same scale).
  * the dense layer stays bf16 (K=361 means DoubleRow padding cancels
    its gain); dense_w is pre-scaled by s_h2/(s_x*s_w1) on the host so
    psum2 lands directly in h2's fp8 scale.
  * h2's hi/lo fp8 split is computed on device: ACT relu -> h2h8, then
    one DVE scalar_tensor_tensor (max, subtract) -> h2l8.
  * the s3 epilogue is a single DVE op: relu(psum3) + x' where the
    residual x' = xh8 + xl8 is reconstructed on the gpsimd engine
    (s_w2*s_h2 == s_x so the units line up exactly); output stores as
    fp16 and the host multiplies by 1/s_x when unmarshalling.  x ships
    once as the fp8 hi/lo pair (2B/elem), partition-major so each load
    is one contiguous >=512B run per partition (under 512B the DMA
    model charges 2x).

Matmul layouts (contraction over partitions, zero on-device transposes):
    s1: ps1[r, d]  += x8[c(kt), r_chunk].T  @ w1[c(kt), d]      (DoubleRow)
    s2: ps2[c, q]  += h1[p_chunk, c_chunk].T @ dw'[p_chunk, q]  (bf16)
    s3: ps3[d, q]  += w2[c(kt), d_chunk].T  @ h2[c(kt), q]      (DoubleRow)

Schedule: 4-deep software pipeline (s1 at step i, s2 at i+1, s3 at i+3 so
the 2-op h2 epilogue chain has two steps of slack); PSUM split 2+4+2 banks
across the three stages; warm-up matmuls on scratch SBUF at t=0 start the
PE p-state ramp (0.65 -> 1.2 -> 2.4 GHz after 3us busy) before the first
DMA lands; x loads ride the sync queue, weights ride gpsimd/SWDGE, output
stores are 8-image SWDGE batches emitted 2 steps late so triggers never
head-of-line-block a queue; the last images store per-dc on sync to cut
the drain tail.
"""

import numpy as np
import ml_dtypes

import concourse.bass as bass
import concourse.mybir as mybir
import concourse.tile as tile
from concourse import bacc
from concourse.bass_utils import run_bass_kernel_spmd

N_CORES = 8
NIMG = 32            # images per core
C = 256
HW = 361             # 19*19
P = 128
EPS = 1e-3
HWP = 368            # HW padded to 16B so DoubleRow ldweights k-tile stride aligns

F32 = mybir.dt.float32
F16 = mybir.dt.float16
BF16 = mybir.dt.bfloat16
FP8 = mybir.dt.float8e4
AF = mybir.ActivationFunctionType
ALU = mybir.AluOpType
DRMODE = mybir.MatmulPerfMode.DoubleRow

E4NP = ml_dtypes.float8_e4m3
BFNP = ml_dtypes.bfloat16

# fp8 weight blob columns: w1h | w1l | w2h | w2l (each [P, 2, 256])
O_W1H, O_W1L, O_W2H, O_W2L = 0, 256, 512, 768
WB8_COLS = 1024

# DMA batches: singles at the edges (short critical path at startup/teardown),
# pairs in steady state
BATCHES = [[0], [1]] + [[i, i + 1] for i in range(2, 30, 2)] + [[30], [31]]
BMAX = 2

_prog_cache = {}

# per-step emission order of matmul groups: (stage, group).  s1 early (its
# relu must land before next step's s2), s2 late (its h2 hi/lo epilogue
# chain gets 2 steps of slack before s3 at step+3 consumes it).
STEP_ORDER = [(1, 0), (1, 1), (1, 2), (2, 0), (3, 0), (3, 1), (2, 1)]
TAIL_ORDER = [(2, 0), (2, 1), (3, 0), (3, 1), (1, 0), (1, 1), (1, 2)]
N_WARM = 24          # warm-up matmuls (PE p-state ramp + startup gap fill)
SGRP = 8             # images per store trigger (amortizes SWDGE gen on Pool)


def build_program(has_b1: bool, has_b2: bool, has_b3: bool, reps: int = 1):
    nc = bacc.Bacc("TRN2", target_bir_lowering=False, debug=False)

    # x8: hi/lo fp8 pair, C-layout with c split as (ktile, partition)
    # partition-major so each batch is one >=512B contiguous run per ci
    x8 = nc.dram_tensor("x8", [P, NIMG, 2, 2, HWP], FP8, kind="ExternalInput").ap()
    wb8 = nc.dram_tensor("wb8", [P, 2, WB8_COLS], FP8, kind="ExternalInput").ap()
    dwb = nc.dram_tensor("dwb", [P, 3, HW], BF16, kind="ExternalInput").ap()
    wf2 = nc.dram_tensor("wf2", [P, 2, C], F16, kind="ExternalInput").ap()
    b1 = b2 = b3 = None
    if has_b1:
        b1 = nc.dram_tensor("b1", [P, 3 * C], F32, kind="ExternalInput").ap()
    if has_b2:
        b2 = nc.dram_tensor("b2", [P, 2, HW], F32, kind="ExternalInput").ap()
    if has_b3:
        b3 = nc.dram_tensor("b3", [2, P], F32, kind="ExternalInput").ap()
    yc = nc.dram_tensor("yc", [NIMG, 2, P, HW], F16, kind="ExternalOutput").ap()

    batch_of = {}
    for bi, imgs in enumerate(BATCHES):
        for k, i in enumerate(imgs):
            batch_of[i] = (bi, k)

    with tile.TileContext(nc) as tc:
        with (
            tc.tile_pool(name="const", bufs=1) as cpool,
            tc.tile_pool(name="x8p", bufs=9) as x8_pool,
            tc.tile_pool(name="xrc", bufs=8) as xrec_pool,
            tc.tile_pool(name="h1", bufs=4) as h1_pool,
            tc.tile_pool(name="h2", bufs=10) as h2_pool,
            tc.tile_pool(name="yo", bufs=2) as yo_pool,
            tc.tile_pool(name="ps1", bufs=1, space="PSUM") as ps1_pool,
            tc.tile_pool(name="ps2", bufs=2, space="PSUM") as ps2_pool,
            tc.tile_pool(name="ps3", bufs=1, space="PSUM") as ps3_pool,
        ):
            # --- warm-up: start the PE ramp before any DMA lands ---------
            warm = cpool.tile([P, P], BF16)
            nc.vector.memset(warm[:], 0.0)
            pw = ps3_pool.tile([P, 1024], F32, tag="ps3", name="pw")
            for wi in range(N_WARM):
                nc.tensor.matmul(
                    pw[:, :P], warm[:], warm[:],
                    start=True, stop=True, skip_group_check=True)

            # --- constants ----------------------------------------------
            # weight blobs ride the (otherwise idle at startup) gpsimd/SWDGE
            # queue; dw goes first on the scalar queue.  This keeps each
            # queue's ~1.3us per-DMA config cost off the critical path.
            wsb8 = cpool.tile([P, 2, WB8_COLS], FP8)
            nc.gpsimd.dma_start(wsb8[:, :, : 2 * C], wb8[:, :, : 2 * C])
            nc.gpsimd.dma_start(wsb8[:, :, 2 * C :], wb8[:, :, 2 * C :])
            dwsb = cpool.tile([P, 3, HW], BF16)
            nc.scalar.dma_start(dwsb[:], dwb)
            # fp16 copy of w2 for the tail images' plain-matmul s3 path
            wf2sb = cpool.tile([P, 2, C], F16)
            nc.scalar.dma_start(wf2sb[:], wf2)

            def w1h_ap():
                return wsb8[:, :, O_W1H : O_W1H + C]

            def w1l_ap():
                return wsb8[:, :, O_W1L : O_W1L + C]

            def w2_ap(hi, dc):
                o = (O_W2H if hi else O_W2L) + dc * P
                return wsb8[:, :, o : o + P]

            b1sb = b2sb = b3sb = None
            if has_b1:
                b1sb = cpool.tile([P, 3 * C], F32)
                nc.sync.dma_start(b1sb[:], b1)
            if has_b2:
                b2sb = cpool.tile([P, 2, HW], F32)
                nc.sync.dma_start(b2sb[:], b2)
            if has_b3:
                b3sb = cpool.tile([P, 2], F32)
                nc.sync.dma_start(b3sb[:], b3.rearrange("co ci -> ci co"))

            def emit_load(bi):
                imgs = BATCHES[bi]
                nb = len(imgs)
                i0 = imgs[0]
                # tile dims: (ci, img, ktile, hi/lo, q) so the DoubleRow
                # lhsT k-tile stride is 736B (16B-aligned) and the DMA is one
                # contiguous 2944B run per partition (no <512B descriptor
                # penalty)
                x8t = x8_pool.tile([P, BMAX, 2, 2, HWP], FP8, tag="x8", name="x8t")
                nc.sync.dma_start(x8t[:, :nb], x8[:, i0 : i0 + nb])
                return x8t

            def emit_s1_group(i, k, x8t, rc, h1, pss):
                m = 128 if rc < 2 else 105
                ps = pss["s1"]
                out = ps[:m, rc * C : rc * C + C]
                lo = rc * 128
                xh = x8t[:, k, :, 0, lo : lo + m]
                xl = x8t[:, k, :, 1, lo : lo + m]
                nc.tensor.matmul(out, xh, w1h_ap(), start=True, stop=False,
                                 perf_mode=DRMODE)
                nc.tensor.matmul(out, xh, w1l_ap(), start=False, stop=False,
                                 perf_mode=DRMODE)
                nc.tensor.matmul(out, xl, w1h_ap(), start=False, stop=True,
                                 perf_mode=DRMODE)
                if rc < 2:
                    return
                # fused epilogue over all three rc slices
                if b1sb is not None:
                    nc.vector.scalar_tensor_tensor(
                        ps[:, : 3 * C], ps[:, : 3 * C], 0.0, b1sb[:],
                        ALU.bypass, ALU.add)
                nc.scalar.activation(
                    h1[:].rearrange("p a b -> p (a b)"), ps[:, : 3 * C], AF.Relu)

            def emit_s2_group(i, h1, cc, h2h, h2l, pss, split_epi=False):
                ps = pss["s2"]
                out = ps[:, cc * 512 : cc * 512 + HW]
                for pc in range(3):
                    kk = 128 if pc < 2 else 105
                    nc.tensor.matmul(
                        out,
                        h1[:kk, pc, cc * 128 : (cc + 1) * 128],
                        dwsb[:kk, pc, :],
                        start=(pc == 0),
                        stop=(pc == 2),
                    )
                if split_epi:
                    # tail images: h2 goes to a single fused fp16 tensor (no
                    # hi/lo split, one ACT op) so the drain-critical chain is
                    # s2 -> one relu -> plain fp16 s3, with DVE free for the
                    # final epilogues
                    if cc == 0:
                        return
                    psv2 = ps.rearrange("p (c x) -> p c x", c=2)[:, :, :HW]
                    if b2sb is not None:
                        nc.vector.scalar_tensor_tensor(
                            psv2, psv2, 0.0, b2sb[:], ALU.bypass, ALU.add)
                    nc.scalar.activation(h2h[:], psv2, AF.Relu)
                    return
                if cc == 0:
                    return
                psv = ps.rearrange("p (c x) -> p c x", c=2)[:, :, :HW]
                if b2sb is not None:
                    nc.vector.scalar_tensor_tensor(
                        psv, psv, 0.0, b2sb[:], ALU.bypass, ALU.add)
                # hi/lo fp8 split of h2: relu -> hi8 on ACT, then one DVE
                # scalar_tensor_tensor (max, subtract) -> lo8
                nc.scalar.activation(h2h[:], psv, AF.Relu)
                nc.vector.scalar_tensor_tensor(
                    h2l[:], psv, 0.0, h2h[:], ALU.max, ALU.subtract)

            def emit_s3_group(i, k, xrec, yo, h2h, h2l, dc, pss):
                ps = pss["s3"]
                slot = i % SGRP
                out = ps[:, dc * 512 : dc * 512 + HW]
                if h2l is None:
                    # tail image: plain fp16 matmuls off the unsplit h2
                    for kt in range(2):
                        nc.tensor.matmul(
                            out, wf2sb[:, kt, dc * P : dc * P + P],
                            h2h[:, kt, :], start=(kt == 0), stop=(kt == 1))
                else:
                    nc.tensor.matmul(out, w2_ap(True, dc), h2h[:], start=True,
                                     stop=False, perf_mode=DRMODE)
                    nc.tensor.matmul(out, w2_ap(True, dc), h2l[:], start=False,
                                     stop=False, perf_mode=DRMODE)
                    nc.tensor.matmul(out, w2_ap(False, dc), h2h[:], start=False,
                                     stop=True, perf_mode=DRMODE)
                if i >= NIMG - 2 and b3sb is None:
                    # tail: per-dc DVE epilogue, one store per image
                    pscc = ps[:, dc * 512 : dc * 512 + HW]
                    nc.vector.scalar_tensor_tensor(
                        yo[:, slot, dc, :], pscc, 0.0, xrec[:, dc, :],
                        ALU.max, ALU.add)
                    if i == NIMG - 1:
                        # final image: per-dc stores on separate queues so
                        # dc0's chain starts during dc1's epilogue
                        q = nc.scalar if dc == 1 else nc.sync
                        q.dma_start(yc[i, dc], yo[:, slot, dc, :])
                    elif dc == 1:
                        nc.sync.dma_start(
                            yc[i].rearrange("co ci q -> ci co q"),
                            yo[:, slot])
                    return
                if dc == 0:
                    return
                psv = ps.rearrange("p (c x) -> p c x", c=2)[:, :, :HW]
                xap = xrec[:]
                if b3sb is not None:
                    for d2 in range(2):
                        nc.scalar.activation(
                            yo[:, slot, d2, :], psv[:, d2, :], AF.Relu,
                            bias=b3sb[:, d2 : d2 + 1])
                    nc.vector.tensor_add(yo[:, slot, :, :], yo[:, slot, :, :], xap)
                    if i >= NIMG - 2:
                        nc.sync.dma_start(
                            yc[i].rearrange("co ci q -> ci co q"), yo[:, slot])
                else:
                    # single fused DVE epilogue: relu(psum3) + x'
                    nc.vector.scalar_tensor_tensor(
                        yo[:, slot, :, :], psv, 0.0, xap, ALU.max, ALU.add)

            def emit_store(g0, yo, n):
                # SWDGE store, emitted well after the data is written so the
                # trigger never head-of-line-blocks Pool's residual adds
                s0 = g0 % SGRP
                nc.gpsimd.dma_start(
                    yc[g0 : g0 + n].rearrange("n co ci q -> ci n co q"),
                    yo[:, s0 : s0 + n])

            def body():
                # software pipeline: s1(i) | s2(i-1) | s3(i-2), interleaved at
                # matmul-group granularity so PSUM slot releases stagger
                xs, h1s, h2s, yos, xrecs = {}, {}, {}, {}, {}

                def load_batch(bi):
                    x8t = emit_load(bi)
                    for k, i in enumerate(BATCHES[bi]):
                        xs[i] = (x8t, k)

                loaded = 0
                for pb in range(4):
                    load_batch(pb)
                    loaded += 1
                for step in range(NIMG + 3):
                    if step % 2 == 0 and loaded < len(BATCHES):
                        load_batch(loaded)
                        loaded += 1
                    i1 = step if step < NIMG else None
                    i2 = step - 1 if 1 <= step <= NIMG else None
                    i3 = step - 3 if step >= 3 else None
                    pss = {}
                    if i1 is not None:
                        pss["s1"] = ps1_pool.tile([P, 768], F32, tag="ps1", name="ps1")
                    if i3 is not None:
                        # last image borrows a (by then idle) ps2 slot so its
                        # matmuls don't WAR-wait on img30's epilogue
                        pool3 = ps2_pool if i3 == NIMG - 1 else ps3_pool
                        tag3 = "ps2" if i3 == NIMG - 1 else "ps3"
                        pss["s3"] = pool3.tile([P, 1024], F32, tag=tag3, name="ps3")
                    if i2 is not None:
                        pss["s2"] = ps2_pool.tile([P, 1024], F32, tag="ps2", name="ps2")
                    if i1 is not None:
                        h1s[i1] = h1_pool.tile([P, 3, C], BF16, tag="h1", name="h1")
                    if i1 is not None:
                        # residual reconstruction on the otherwise idle
                        # gpsimd engine: xrec = xh8 + xl8 (= s_x * x exactly
                        # in epilogue units, since s_w2*s_h2 == s_x)
                        x8t, k = xs[i1]
                        xrec = xrec_pool.tile([P, 2, HW], F16, tag="xc", name="xrec")
                        nc.gpsimd.tensor_add(
                            xrec[:], x8t[:, k, :, 0, :HW], x8t[:, k, :, 1, :HW])
                        xrecs[i1] = xrec
                    if i3 is not None and i3 % SGRP == 0:
                        yos[i3 // SGRP] = yo_pool.tile(
                            [P, SGRP, 2, HW], F16, tag="yo", name="yo")
                    if i2 is not None:
                        if i2 >= NIMG - 3:
                            h2s[i2] = (
                                xrec_pool.tile([P, 2, HW], F16, tag="xc",
                                               name="h2f"),
                                None,
                            )
                        else:
                            h2s[i2] = (
                                h2_pool.tile([P, 2, HW], FP8, tag="h2h", name="h2h"),
                                h2_pool.tile([P, 2, HW], FP8, tag="h2l", name="h2l"),
                            )
                    order = STEP_ORDER if step < NIMG else TAIL_ORDER
                    for stg, g in order:
                        if stg == 1 and i1 is not None:
                            x8t, k = xs[i1]
                            emit_s1_group(i1, k, x8t, g, h1s[i1], pss)
                        elif stg == 3 and i3 is not None:
                            emit_s3_group(i3, 0, xrecs[i3], yos[i3 // SGRP],
                                          h2s[i3][0], h2s[i3][1], g, pss)
                        elif stg == 2 and i2 is not None:
                            emit_s2_group(i2, h1s[i2], g,
                                          h2s[i2][0], h2s[i2][1], pss,
                                          split_epi=(i2 >= NIMG - 3))
                    if i2 is not None:
                        h1s.pop(i2)
                    if i3 is not None:
                        h2s.pop(i3)
                        xrecs.pop(i3)
                        # delayed stores: 8-img groups for 0..23 (2 steps
                        # after the group's last s3), pairs for 24..29
                        if i3 in (9, 17, 25):
                            g0 = i3 - 9
                            emit_store(g0, yos[g0 // SGRP], SGRP)
                        elif i3 == 28:
                            emit_store(24, yos[3], 2)
                        elif i3 == 30:
                            emit_store(26, yos[3], 2)
                        elif i3 == NIMG - 1:
                            emit_store(28, yos[3], 2)

            if reps == 1:
                body()
            else:
                with tc.For_i(0, reps, 1):
                    body()

    nc.compile()
    return nc


def _get_program(key):
    if key not in _prog_cache:
        _prog_cache[key] = build_program(*key)
    return _prog_cache[key]


def _pow2floor(v):
    return 2.0 ** np.floor(np.log2(max(float(v), 1e-30)))


def _q8(a):
    return np.clip(a, -240.0, 240.0).astype(E4NP)


def _marshal(x, conv1_w, conv1_b, bn1_mean, bn1_var, bn1_beta,
             dense_w, dense_b, conv2_w, conv2_b, bn2_mean, bn2_var, bn2_beta):
    n = x.shape[0]
    rs1 = 1.0 / np.sqrt(bn1_var.astype(np.float64) + EPS)
    rs2 = 1.0 / np.sqrt(bn2_var.astype(np.float64) + EPS)
    w1f = conv1_w.astype(np.float64) * rs1[None, :]
    w2f = conv2_w.astype(np.float64) * rs2[None, :]
    b1f = (conv1_b - bn1_mean).astype(np.float64) * rs1 + bn1_beta
    b2f = dense_b.astype(np.float64)
    b3f = (conv2_b - bn2_mean).astype(np.float64) * rs2 + bn2_beta
    has_b1 = bool(np.any(b1f != 0.0))
    has_b2 = bool(np.any(b2f != 0.0))
    has_b3 = bool(np.any(b3f != 0.0))

    s_x = _pow2floor(224.0 / max(np.abs(x).max(), 1e-20))
    s_w1 = _pow2floor(224.0 / max(np.abs(w1f).max(), 1e-20))
    s_h2 = 4.0
    # s_w2*s_h2 == s_x so the residual in epilogue units is exactly xh8+xl8
    # (reconstructed on device); w2 values are small so s_x/4 keeps them
    # well inside e4m3 range
    s_w2 = s_x / s_h2
    k_out = 1.0 / (s_w2 * s_h2)

    # x in C-layout [n, C, HW], C split as (ktile, partition)
    xq = np.ascontiguousarray(
        x.reshape(n, HW, C).transpose(0, 2, 1)).astype(np.float32)
    xh8 = _q8(xq * np.float32(s_x))
    xl8 = _q8(xq * np.float32(s_x) - xh8.astype(np.float32))
    x8_all = np.stack([xh8, xl8], axis=1)        # [n, t, C, HW]
    x8_all = x8_all.reshape(N_CORES, NIMG, 2, 2, P, HW)
    x8_pad = np.zeros((N_CORES, NIMG, 2, 2, P, HWP), E4NP)
    x8_pad[..., :HW] = x8_all
    # -> [core, ci, img, kt, t, q]
    x8_all = np.ascontiguousarray(x8_pad.transpose(0, 4, 1, 3, 2, 5))

    # fp8 weight blob [P, 2, 1024]: w1h | w1l | w2h | w2l
    blob = np.zeros((P, 2, WB8_COLS), E4NP)
    w1h = _q8(w1f * s_w1)
    w1l = _q8(w1f * s_w1 - w1h.astype(np.float64))
    w2h = _q8(w2f * s_w2)
    w2l = _q8(w2f * s_w2 - w2h.astype(np.float64))
    for kt in range(2):
        blob[:, kt, O_W1H : O_W1H + C] = w1h[kt * P : (kt + 1) * P]
        blob[:, kt, O_W1L : O_W1L + C] = w1l[kt * P : (kt + 1) * P]
        blob[:, kt, O_W2H : O_W2H + C] = w2h[kt * P : (kt + 1) * P]
        blob[:, kt, O_W2L : O_W2L + C] = w2l[kt * P : (kt + 1) * P]

    wf2 = np.ascontiguousarray(
        (w2f * s_w2).reshape(2, P, C).transpose(1, 0, 2)).astype(np.float16)

    # dense weight, pre-scaled so psum2 = s_h2 * h2pre, padded to 3 k-tiles
    dwf = dense_w.astype(np.float64) * (s_h2 / (s_x * s_w1))
    dwp = np.zeros((3 * P, HW), np.float64)
    dwp[:HW] = dwf
    dwb = np.ascontiguousarray(dwp.reshape(3, P, HW).transpose(1, 0, 2)).astype(BFNP)

    in_maps = []
    for c in range(N_CORES):
        m = {"x8": x8_all[c], "wb8": blob, "dwb": dwb, "wf2": wf2}
        if has_b1:
            m["b1"] = np.ascontiguousarray(np.broadcast_to(
                np.tile(b1f * (s_x * s_w1), 3).astype(np.float32), (P, 3 * C)))
        if has_b2:
            m["b2"] = np.ascontiguousarray(np.broadcast_to(
                (b2f * s_h2).astype(np.float32), (P, 2, HW)))
        if has_b3:
            m["b3"] = np.ascontiguousarray(
                (b3f * (s_w2 * s_h2)).astype(np.float32).reshape(2, P))
        in_maps.append(m)
    return (has_b1, has_b2, has_b3), in_maps, k_out


def _unmarshal(results, n, h, w, k_out):
    y = np.stack([results[c]["yc"] for c in range(N_CORES)])
    y = y.astype(np.float32).reshape(n, C, HW).transpose(0, 2, 1)
    y = y * np.float32(k_out)
    return np.ascontiguousarray(y.reshape(n, h, w, C))


def kernel(x, conv1_w, conv1_b, bn1_mean, bn1_var, bn1_beta,
           dense_w, dense_b, conv2_w, conv2_b, bn2_mean, bn2_var, bn2_beta):
    n, h, w, _ = x.shape
    flags, in_maps, k_out = _marshal(
        x, conv1_w, conv1_b, bn1_mean, bn1_var, bn1_beta,
        dense_w, dense_b, conv2_w, conv2_b, bn2_mean, bn2_var, bn2_beta)
    nc = _get_program((*flags, 1))
    res = run_bass_kernel_spmd(nc, in_maps, list(range(N_CORES)))
    return _unmarshal(res.results, n, h, w, k_out)
